# revision 6
# baseline (speedup 1.0000x reference)
"""Trainium2 Bass/Tile kernel for nn_MultiHeadAttention (B=4, S=2048, D=1024,
H=16, Dh=64, fp32), SPMD across 8 NeuronCores.

Sharding: core c -> batch c//2, head-half c%2 (8 heads per core).
Host pre-transposes each batch slice to [D, S] and casts to bf16, so the
device needs no transposes: QK projections produce Q^T/K^T [feat, tok]
directly (weight as stationary), the V projection produces V [tok, feat]
with an appended ones-column, scores come out as scores^T [k, q] (two
heads row-packed on the 128-wide contraction via tile_position), exp runs
on the scalar engine with the 1/sqrt(Dh) scale folded in (scores are
bounded ~±3, so no max-subtraction is needed), and the PV matmul uses
V as the stationary operand, yielding out^T plus the softmax denominator
for free from the ones column.  The host divides by the denominator,
adds the V bias (exact because softmax rows sum to 1), transposes, and
reassembles the full [4, 2048, 1024] fp32 output.

PSUM budget (8 banks): scores 2 tiles x 2 banks, PV accumulator 2 banks,
projection slot 2 banks.  The exp pool holds 20 k-tiles so the scalar
engine can run ahead while the V projection finishes.
"""

import numpy as np
import ml_dtypes

import concourse.bacc as bacc
import concourse.tile as tile
from concourse import mybir
from concourse.bass_utils import run_bass_kernel_spmd

F32 = mybir.dt.float32
BF16 = mybir.dt.bfloat16
_BF = ml_dtypes.bfloat16

B, S, D, H, DH = 4, 2048, 1024, 16, 64
HH = 8          # heads per core
NP = HH // 2    # head pairs per core
JW = HH * DH    # 512 projected features per core
N_CORES = 8


def _build_nc(S=S, qt_size=512, sc_bufs=2, pv_bufs=2, exp_bufs=36, in_bufs=10):
    KT8 = D // 128
    NQT = S // qt_size
    NKT = S // 128
    NTT = S // 128
    TC = 512
    NTC = S // TC

    nc = bacc.Bacc("TRN2", target_bir_lowering=False, debug=False,
                   num_devices=N_CORES)

    qT = nc.declare_dram_parameter("qT", [D, S], BF16, isOutput=False)
    kT = nc.declare_dram_parameter("kT", [D, S], BF16, isOutput=False)
    vT = nc.declare_dram_parameter("vT", [D, S], BF16, isOutput=False)
    wq = nc.declare_dram_parameter("wq", [D, JW], BF16, isOutput=False)
    wk = nc.declare_dram_parameter("wk", [D, JW], BF16, isOutput=False)
    wv = nc.declare_dram_parameter("wv", [D, JW], BF16, isOutput=False)
    bq = nc.declare_dram_parameter("bq", [JW], F32, isOutput=False)
    bk = nc.declare_dram_parameter("bk", [JW], F32, isOutput=False)
    numT = nc.declare_dram_parameter("numT", [JW, S], F32, isOutput=True)
    den = nc.declare_dram_parameter("den", [HH, S], F32, isOutput=True)
    w_dram = {"wq": wq, "wk": wk, "wv": wv}
    in_dram = {"q": qT, "k": kT, "v": vT}

    with tile.TileContext(nc) as tc:
        with (
            tc.tile_pool(name="consts", bufs=1) as consts,
            tc.tile_pool(name="persist", bufs=1) as persist,
            tc.tile_pool(name="ins", bufs=in_bufs) as ins,
            tc.tile_pool(name="exps", bufs=exp_bufs) as exps,
            tc.tile_pool(name="ostage", bufs=4) as ostage,
            tc.tile_pool(name="scps", bufs=sc_bufs, space="PSUM") as scps,
            tc.tile_pool(name="pvps", bufs=pv_bufs, space="PSUM") as pvps,
            tc.tile_pool(name="prps", bufs=1, space="PSUM") as prps,
        ):
            w_sb = {}

            def load_w(name):
                t = consts.tile([128, KT8, JW], BF16, tag=name)
                src_r = w_dram[name].ap().rearrange("(kt p) j -> p kt j", p=128)
                for kt in range(KT8):
                    nc.scalar.dma_start(out=t[:, kt, :], in_=src_r[:, kt, :])
                w_sb[name] = t

            def load_bias(name, src):
                t = consts.tile([128, NP], F32, tag=name)
                nc.sync.dma_start(
                    out=t[:], in_=src.ap().rearrange("(pr j) -> j pr", j=128))
                return t

            QT_sb = persist.tile([128, NP, S], BF16, tag="QT")
            KT_sb = persist.tile([128, NP, S], BF16, tag="KT")
            V_aug = persist.tile([128, NTT, HH, 65], BF16, tag="Vaug")

            def load_input(name, kt):
                t = ins.tile([128, S], BF16, tag="in")
                eng = nc.gpsimd if name == "q" else nc.sync
                eng.dma_start(
                    out=t[:], in_=in_dram[name].ap()[kt * 128:(kt + 1) * 128, :])
                return t

            def proj_qk(pair):
                for name, wname, bias, dst in (
                        ("k", "wk", bias_k, KT_sb), ("q", "wq", bias_q, QT_sb)):
                    if wname not in w_sb:
                        load_w(wname)
                    tiles = [load_input(name, kt) for kt in range(KT8)]
                    for s in range(NTC // 2):
                        ps = prps.tile([128, 2, TC], F32, tag="pr")
                        for kt in range(KT8):
                            lw = w_sb[wname][:, kt, pair * 128:(pair + 1) * 128]
                            for i in range(2):
                                tc0 = (2 * s + i) * TC
                                nc.tensor.matmul(
                                    ps[:, i, :], lw, tiles[kt][:, tc0:tc0 + TC],
                                    start=(kt == 0), stop=(kt == KT8 - 1))
                        nc.vector.tensor_scalar_add(
                            dst[:, pair, 2 * s * TC:(2 * s + 2) * TC],
                            ps[:].rearrange("p a b -> p (a b)"),
                            bias[:, pair:pair + 1])

            def proj_v():
                load_w("wv")
                nc.vector.memset(V_aug[:, :, :, 64:65], 1.0)
                tiles = [load_input("v", kt) for kt in range(KT8)]
                for s in range(NTT // 2):
                    ps = prps.tile([128, 2, JW], F32, tag="pr")
                    for kt in range(KT8):
                        for i in range(2):
                            tt = 2 * s + i
                            nc.tensor.matmul(
                                ps[:, i, :],
                                tiles[kt][:, tt * 128:(tt + 1) * 128],
                                w_sb["wv"][:, kt, :],
                                start=(kt == 0), stop=(kt == KT8 - 1))
                    nc.vector.tensor_copy(
                        V_aug[:, 2 * s:2 * s + 2, :, 0:64],
                        ps[:].rearrange("p a (h d) -> p a h d", d=64))

            def attn_scores_block(pair, qtp):
                """Scores+exp for the two q-tiles 2*qtp, 2*qtp+1.  Emission
                order (h0,q0) (h1,q0) (h0,q1) (h1,q1) per k-tile keeps the
                same stationary operand on consecutive same-head matmuls."""
                ets = {}
                for kt in range(NKT):
                    for qi in range(2):
                        q0 = (2 * qtp + qi) * qt_size
                        sc = scps.tile([128, 2, qt_size], F32, tag="sc")
                        for h2 in range(2):
                            nc.tensor.matmul(
                                sc[:, h2, :],
                                KT_sb[h2 * 64:(h2 + 1) * 64, pair,
                                      kt * 128:(kt + 1) * 128],
                                QT_sb[h2 * 64:(h2 + 1) * 64, pair,
                                      q0:q0 + qt_size],
                                start=True, stop=True)
                        et = exps.tile([128, 2, qt_size], BF16, tag="exp")
                        nc.scalar.activation(
                            et[:].rearrange("p a b -> p (a b)"),
                            sc[:].rearrange("p a b -> p (a b)"),
                            mybir.ActivationFunctionType.Exp, scale=0.125)
                        ets[(kt, qi)] = et
                return ets

            def attn_pv_block(pair, qtp, ets):
                """Trailing PV chains; both q-tiles interleaved per head so
                each V stationary is loaded once per (kt, head)."""
                for h2 in range(2):
                    h = pair * 2 + h2
                    pvs = [pvps.tile([65, qt_size], F32, tag="pv",
                                     name=f"pv_{pair}_{qtp}_{h2}_{qi}")
                           for qi in range(2)]
                    for kt in range(NKT):
                        for qi in range(2):
                            nc.tensor.matmul(
                                pvs[qi][:],
                                V_aug[:, kt, h, :],
                                ets[(kt, qi)][:, h2, :],
                                start=(kt == 0), stop=(kt == NKT - 1))
                    for qi in range(2):
                        q0 = (2 * qtp + qi) * qt_size
                        ot = ostage.tile([65, qt_size], F32, tag="ot")
                        nc.vector.tensor_copy(ot[:], pvs[qi][:])
                        nc.sync.dma_start(
                            out=numT.ap()[h * 64:(h + 1) * 64, q0:q0 + qt_size],
                            in_=ot[0:64, :])
                        nc.sync.dma_start(
                            out=den.ap()[h:h + 1, q0:q0 + qt_size],
                            in_=ot[64:65, :])

            def attn_block(pair, qtp):
                attn_pv_block(pair, qtp, attn_scores_block(pair, qtp))

            load_w("wk")
            bias_q = load_bias("bq", bq)
            bias_k = load_bias("bk", bk)
            proj_qk(0)
            ets0 = attn_scores_block(0, 0)
            proj_v()
            attn_pv_block(0, 0, ets0)
            for qtp in range(1, NQT // 2):
                attn_block(0, qtp)
            for pair in range(1, NP):
                proj_qk(pair)
                for qtp in range(NQT // 2):
                    attn_block(pair, qtp)

    nc.compile()
    return nc


_NC_CACHE = {}


def _get_nc():
    if "nc" not in _NC_CACHE:
        _NC_CACHE["nc"] = _build_nc()
    return _NC_CACHE["nc"]


def _make_in_maps(key, value, query, Wq, bq, Wk, bk, Wv):
    in_maps = []
    for c in range(N_CORES):
        b, hh = c // 2, c % 2
        js = slice(hh * JW, (hh + 1) * JW)
        in_maps.append({
            "qT": np.ascontiguousarray(query[b].T).astype(_BF),
            "kT": np.ascontiguousarray(key[b].T).astype(_BF),
            "vT": np.ascontiguousarray(value[b].T).astype(_BF),
            "wq": np.ascontiguousarray(Wq[:, js]).astype(_BF),
            "wk": np.ascontiguousarray(Wk[:, js]).astype(_BF),
            "wv": np.ascontiguousarray(Wv[:, js]).astype(_BF),
            "bq": np.ascontiguousarray(bq[js], dtype=np.float32),
            "bk": np.ascontiguousarray(bk[js], dtype=np.float32),
        })
    return in_maps


def _assemble(results, bv):
    out = np.empty((B, S, H * DH), np.float32)
    for c in range(N_CORES):
        b, hh = c // 2, c % 2
        numT = results[c]["numT"]
        den = results[c]["den"]
        blk = numT.reshape(HH, DH, S) / den[:, None, :]
        out[b, :, hh * JW:(hh + 1) * JW] = (
            blk.reshape(JW, S).T + bv[hh * JW:(hh + 1) * JW])
    return out


def kernel(key, value, query, Wq, bq, Wk, bk, Wv, bv, **_run_kwargs):
    key = np.asarray(key, np.float32)
    value = np.asarray(value, np.float32)
    query = np.asarray(query, np.float32)
    nc = _get_nc()
    in_maps = _make_in_maps(key, value, query,
                            np.asarray(Wq, np.float32), np.asarray(bq, np.float32),
                            np.asarray(Wk, np.float32), np.asarray(bk, np.float32),
                            np.asarray(Wv, np.float32))
    res = run_bass_kernel_spmd(nc, in_maps, list(range(N_CORES)), **_run_kwargs)
    out = _assemble(res.results, np.asarray(bv, np.float32))
    if _run_kwargs:
        kernel.last_result = res
    return out


# revision 7
# speedup vs baseline: 1.1019x; 1.1019x over previous
"""Trainium2 Bass/Tile kernel for nn_MultiHeadAttention (B=4, S=2048, D=1024,
H=16, Dh=64, fp32), SPMD across 8 NeuronCores.

Sharding: core c -> batch c//2, head-half c%2 (8 heads per core).
Host pre-transposes each batch slice to [D, S] and casts to bf16, so the
device needs no transposes: QK projections produce Q^T/K^T [feat, tok]
directly (weight as stationary), the V projection produces V [tok, feat]
with an appended ones-column, scores come out as scores^T [k, q] (two
heads row-packed on the 128-wide contraction via tile_position), exp runs
on the scalar engine with the 1/sqrt(Dh) scale folded in (scores are
bounded ~±3, so no max-subtraction is needed), and the PV matmul uses
V as the stationary operand, yielding out^T plus the softmax denominator
for free from the ones column.  The host divides by the denominator,
adds the V bias (exact because softmax rows sum to 1), transposes, and
reassembles the full [4, 2048, 1024] fp32 output.

PSUM budget (8 banks): scores 2 tiles x 2 banks, PV accumulator 2 banks,
projection slot 2 banks.  The exp pool holds 20 k-tiles so the scalar
engine can run ahead while the V projection finishes.
"""

import numpy as np
import ml_dtypes

import concourse.bacc as bacc
import concourse.tile as tile
from concourse import mybir
from concourse.bass_utils import run_bass_kernel_spmd

F32 = mybir.dt.float32
BF16 = mybir.dt.bfloat16
_BF = ml_dtypes.bfloat16

B, S, D, H, DH = 4, 2048, 1024, 16, 64
HH = 8          # heads per core
NP = HH // 2    # head pairs per core
JW = HH * DH    # 512 projected features per core
N_CORES = 8


def _build_nc(S=S, qt_size=512, sc_bufs=2, pv_bufs=2, exp_bufs=32, in_bufs=10):
    KT8 = D // 128
    NQT = S // qt_size
    NKT = S // 128
    NTT = S // 128
    TC = 512
    NTC = S // TC

    nc = bacc.Bacc("TRN2", target_bir_lowering=False, debug=False,
                   num_devices=N_CORES)

    qT = nc.declare_dram_parameter("qT", [D, S], BF16, isOutput=False)
    kT = nc.declare_dram_parameter("kT", [D, S], BF16, isOutput=False)
    vT = nc.declare_dram_parameter("vT", [D, S], BF16, isOutput=False)
    wq = nc.declare_dram_parameter("wq", [D, JW], BF16, isOutput=False)
    wk = nc.declare_dram_parameter("wk", [D, JW], BF16, isOutput=False)
    wv = nc.declare_dram_parameter("wv", [D, JW], BF16, isOutput=False)
    bq = nc.declare_dram_parameter("bq", [JW], F32, isOutput=False)
    bk = nc.declare_dram_parameter("bk", [JW], F32, isOutput=False)
    numT = nc.declare_dram_parameter("numT", [JW, S], F32, isOutput=True)
    den = nc.declare_dram_parameter("den", [HH, S], F32, isOutput=True)
    w_dram = {"wq": wq, "wk": wk, "wv": wv}
    in_dram = {"q": qT, "k": kT, "v": vT}

    with tile.TileContext(nc) as tc:
        with (
            tc.tile_pool(name="consts", bufs=1) as consts,
            tc.tile_pool(name="persist", bufs=1) as persist,
            tc.tile_pool(name="ins", bufs=in_bufs) as ins,
            tc.tile_pool(name="exps", bufs=exp_bufs) as exps,
            tc.tile_pool(name="ostage", bufs=4) as ostage,
            tc.tile_pool(name="scps", bufs=sc_bufs, space="PSUM") as scps,
            tc.tile_pool(name="pvps", bufs=pv_bufs, space="PSUM") as pvps,
            tc.tile_pool(name="prps", bufs=1, space="PSUM") as prps,
        ):
            w_sb = {}

            def load_w(name):
                t = consts.tile([128, KT8, JW], BF16, tag=name)
                src_r = w_dram[name].ap().rearrange("(kt p) j -> p kt j", p=128)
                for kt in range(KT8):
                    nc.scalar.dma_start(out=t[:, kt, :], in_=src_r[:, kt, :])
                w_sb[name] = t

            def load_bias(name, src):
                t = consts.tile([128, NP], F32, tag=name)
                nc.sync.dma_start(
                    out=t[:], in_=src.ap().rearrange("(pr j) -> j pr", j=128))
                return t

            QT_sb = persist.tile([128, NP, S], BF16, tag="QT")
            KT_sb = persist.tile([128, NP, S], BF16, tag="KT")
            V_aug = persist.tile([128, NTT, HH, 65], BF16, tag="Vaug")

            def load_input(name, kt):
                t = ins.tile([128, S], BF16, tag="in")
                eng = nc.gpsimd if name == "q" else nc.sync
                eng.dma_start(
                    out=t[:], in_=in_dram[name].ap()[kt * 128:(kt + 1) * 128, :])
                return t

            def proj_qk(pair):
                for name, wname, bias, dst in (
                        ("k", "wk", bias_k, KT_sb), ("q", "wq", bias_q, QT_sb)):
                    if wname not in w_sb:
                        load_w(wname)
                    tiles = [load_input(name, kt) for kt in range(KT8)]
                    for s in range(NTC // 2):
                        ps = prps.tile([128, 2, TC], F32, tag="pr")
                        for kt in range(KT8):
                            lw = w_sb[wname][:, kt, pair * 128:(pair + 1) * 128]
                            for i in range(2):
                                tc0 = (2 * s + i) * TC
                                nc.tensor.matmul(
                                    ps[:, i, :], lw, tiles[kt][:, tc0:tc0 + TC],
                                    start=(kt == 0), stop=(kt == KT8 - 1))
                        nc.vector.tensor_scalar_add(
                            dst[:, pair, 2 * s * TC:(2 * s + 2) * TC],
                            ps[:].rearrange("p a b -> p (a b)"),
                            bias[:, pair:pair + 1])

            def proj_v():
                load_w("wv")
                nc.vector.memset(V_aug[:, :, :, 64:65], 1.0)
                tiles = [load_input("v", kt) for kt in range(KT8)]
                for s in range(NTT // 2):
                    ps = prps.tile([128, 2, JW], F32, tag="pr")
                    for kt in range(KT8):
                        for i in range(2):
                            tt = 2 * s + i
                            nc.tensor.matmul(
                                ps[:, i, :],
                                tiles[kt][:, tt * 128:(tt + 1) * 128],
                                w_sb["wv"][:, kt, :],
                                start=(kt == 0), stop=(kt == KT8 - 1))
                    nc.vector.tensor_copy(
                        V_aug[:, 2 * s:2 * s + 2, :, 0:64],
                        ps[:].rearrange("p a (h d) -> p a h d", d=64))

            def attn_scores(pair, qt):
                """Emit 16 (scores, exp) groups; return the et tiles."""
                q0 = qt * qt_size
                ets = []
                for kt in range(NKT):
                    sc = scps.tile([128, 2, qt_size], F32, tag="sc")
                    for h2 in range(2):
                        nc.tensor.matmul(
                            sc[:, h2, :],
                            KT_sb[h2 * 64:(h2 + 1) * 64, pair,
                                  kt * 128:(kt + 1) * 128],
                            QT_sb[h2 * 64:(h2 + 1) * 64, pair, q0:q0 + qt_size],
                            start=True, stop=True)
                    et = exps.tile([128, 2, qt_size], BF16, tag="exp")
                    nc.scalar.activation(
                        et[:].rearrange("p a b -> p (a b)"),
                        sc[:].rearrange("p a b -> p (a b)"),
                        mybir.ActivationFunctionType.Exp, scale=0.125)
                    ets.append(et)
                return ets

            def attn_pv(pair, qt, ets):
                """Trailing per-head PV chains (1 PSUM bank each, bufs=2)."""
                q0 = qt * qt_size
                for h2 in range(2):
                    h = pair * 2 + h2
                    pv = pvps.tile([65, qt_size], F32, tag="pv")
                    for kt in range(NKT):
                        nc.tensor.matmul(
                            pv[:],
                            V_aug[:, kt, h, :],
                            ets[kt][:, h2, :],
                            start=(kt == 0), stop=(kt == NKT - 1))
                    ot = ostage.tile([65, qt_size], F32, tag="ot")
                    nc.vector.tensor_copy(ot[:], pv[:])
                    nc.sync.dma_start(
                        out=numT.ap()[h * 64:(h + 1) * 64, q0:q0 + qt_size],
                        in_=ot[0:64, :])
                    nc.sync.dma_start(
                        out=den.ap()[h:h + 1, q0:q0 + qt_size],
                        in_=ot[64:65, :])

            def attn_qt(pair, qt):
                attn_pv(pair, qt, attn_scores(pair, qt))

            load_w("wk")
            bias_q = load_bias("bq", bq)
            bias_k = load_bias("bk", bk)
            proj_qk(0)
            ets0 = attn_scores(0, 0)
            proj_v()
            attn_pv(0, 0, ets0)
            for qt in range(1, NQT):
                attn_qt(0, qt)
            for pair in range(1, NP):
                proj_qk(pair)
                for qt in range(NQT):
                    attn_qt(pair, qt)

    nc.compile()
    return nc


_NC_CACHE = {}


def _get_nc():
    if "nc" not in _NC_CACHE:
        _NC_CACHE["nc"] = _build_nc()
    return _NC_CACHE["nc"]


def _make_in_maps(key, value, query, Wq, bq, Wk, bk, Wv):
    in_maps = []
    for c in range(N_CORES):
        b, hh = c // 2, c % 2
        js = slice(hh * JW, (hh + 1) * JW)
        in_maps.append({
            "qT": np.ascontiguousarray(query[b].T).astype(_BF),
            "kT": np.ascontiguousarray(key[b].T).astype(_BF),
            "vT": np.ascontiguousarray(value[b].T).astype(_BF),
            "wq": np.ascontiguousarray(Wq[:, js]).astype(_BF),
            "wk": np.ascontiguousarray(Wk[:, js]).astype(_BF),
            "wv": np.ascontiguousarray(Wv[:, js]).astype(_BF),
            "bq": np.ascontiguousarray(bq[js], dtype=np.float32),
            "bk": np.ascontiguousarray(bk[js], dtype=np.float32),
        })
    return in_maps


def _assemble(results, bv):
    out = np.empty((B, S, H * DH), np.float32)
    for c in range(N_CORES):
        b, hh = c // 2, c % 2
        numT = results[c]["numT"]
        den = results[c]["den"]
        blk = numT.reshape(HH, DH, S) / den[:, None, :]
        out[b, :, hh * JW:(hh + 1) * JW] = (
            blk.reshape(JW, S).T + bv[hh * JW:(hh + 1) * JW])
    return out


def kernel(key, value, query, Wq, bq, Wk, bk, Wv, bv, **_run_kwargs):
    key = np.asarray(key, np.float32)
    value = np.asarray(value, np.float32)
    query = np.asarray(query, np.float32)
    nc = _get_nc()
    in_maps = _make_in_maps(key, value, query,
                            np.asarray(Wq, np.float32), np.asarray(bq, np.float32),
                            np.asarray(Wk, np.float32), np.asarray(bk, np.float32),
                            np.asarray(Wv, np.float32))
    res = run_bass_kernel_spmd(nc, in_maps, list(range(N_CORES)), **_run_kwargs)
    out = _assemble(res.results, np.asarray(bv, np.float32))
    if _run_kwargs:
        kernel.last_result = res
    return out


# revision 8
# speedup vs baseline: 1.1091x; 1.0066x over previous
"""Trainium2 Bass/Tile kernel for nn_MultiHeadAttention (B=4, S=2048, D=1024,
H=16, Dh=64, fp32), SPMD across 8 NeuronCores.

Sharding: core c -> batch c//2, head-half c%2 (8 heads per core).
Host pre-transposes each batch slice to [D, S] and casts to bf16, so the
device needs no transposes: QK projections produce Q^T/K^T [feat, tok]
directly (weight as stationary), the V projection produces V [tok, feat]
with an appended ones-column, scores come out as scores^T [k, q] (two
heads row-packed on the 128-wide contraction via tile_position), exp runs
on the scalar engine with the 1/sqrt(Dh) scale folded in (scores are
bounded ~±3, so no max-subtraction is needed), and the PV matmul uses
V as the stationary operand, yielding out^T plus the softmax denominator
for free from the ones column.  The host divides by the denominator,
adds the V bias (exact because softmax rows sum to 1), transposes, and
reassembles the full [4, 2048, 1024] fp32 output.

PSUM budget (8 banks): scores 2 tiles x 2 banks, PV accumulator 2 banks,
projection slot 2 banks.  The exp pool holds 20 k-tiles so the scalar
engine can run ahead while the V projection finishes.
"""

import numpy as np
import ml_dtypes

import concourse.bacc as bacc
import concourse.tile as tile
from concourse import mybir
from concourse.bass_utils import run_bass_kernel_spmd

F32 = mybir.dt.float32
BF16 = mybir.dt.bfloat16
_BF = ml_dtypes.bfloat16

B, S, D, H, DH = 4, 2048, 1024, 16, 64
HH = 8          # heads per core
NP = HH // 2    # head pairs per core
JW = HH * DH    # 512 projected features per core
N_CORES = 8


def _build_nc(S=S, qt_size=512, sc_bufs=2, pv_bufs=2, exp_bufs=32, in_bufs=10):
    KT8 = D // 128
    NQT = S // qt_size
    NKT = S // 128
    NTT = S // 128
    TC = 512
    NTC = S // TC

    nc = bacc.Bacc("TRN2", target_bir_lowering=False, debug=False,
                   num_devices=N_CORES)

    qT = nc.declare_dram_parameter("qT", [D, S], BF16, isOutput=False)
    kT = nc.declare_dram_parameter("kT", [D, S], BF16, isOutput=False)
    vT = nc.declare_dram_parameter("vT", [D, S], BF16, isOutput=False)
    wq = nc.declare_dram_parameter("wq", [D, JW], BF16, isOutput=False)
    wk = nc.declare_dram_parameter("wk", [D, JW], BF16, isOutput=False)
    wv = nc.declare_dram_parameter("wv", [D, JW], BF16, isOutput=False)
    bq = nc.declare_dram_parameter("bq", [JW], F32, isOutput=False)
    bk = nc.declare_dram_parameter("bk", [JW], F32, isOutput=False)
    numT = nc.declare_dram_parameter("numT", [JW, S], F32, isOutput=True)
    den = nc.declare_dram_parameter("den", [HH, S], F32, isOutput=True)
    w_dram = {"wq": wq, "wk": wk, "wv": wv}
    in_dram = {"q": qT, "k": kT, "v": vT}

    with tile.TileContext(nc) as tc:
        with (
            tc.tile_pool(name="consts", bufs=1) as consts,
            tc.tile_pool(name="persist", bufs=1) as persist,
            tc.tile_pool(name="ins", bufs=in_bufs) as ins,
            tc.tile_pool(name="exps", bufs=exp_bufs) as exps,
            tc.tile_pool(name="ostage", bufs=4) as ostage,
            tc.tile_pool(name="scps", bufs=sc_bufs, space="PSUM") as scps,
            tc.tile_pool(name="pvps", bufs=pv_bufs, space="PSUM") as pvps,
            tc.tile_pool(name="prps", bufs=1, space="PSUM") as prps,
        ):
            w_sb = {}

            def load_w(name):
                t = consts.tile([128, KT8, JW], BF16, tag=name)
                src_r = w_dram[name].ap().rearrange("(kt p) j -> p kt j", p=128)
                for kt in range(KT8):
                    nc.gpsimd.dma_start(out=t[:, kt, :], in_=src_r[:, kt, :])
                w_sb[name] = t

            def load_bias(name, src):
                t = consts.tile([128, NP], F32, tag=name)
                nc.sync.dma_start(
                    out=t[:], in_=src.ap().rearrange("(pr j) -> j pr", j=128))
                return t

            QT_sb = persist.tile([128, NP, S], BF16, tag="QT")
            KT_sb = persist.tile([128, NP, S], BF16, tag="KT")
            V_aug = persist.tile([128, NTT, HH, 65], BF16, tag="Vaug")

            def load_input(name, kt):
                t = ins.tile([128, S], BF16, tag="in")
                eng = nc.gpsimd if name == "q" else nc.sync
                eng.dma_start(
                    out=t[:], in_=in_dram[name].ap()[kt * 128:(kt + 1) * 128, :])
                return t

            def proj_qk(pair):
                for name, wname, bias, dst in (
                        ("k", "wk", bias_k, KT_sb), ("q", "wq", bias_q, QT_sb)):
                    if wname not in w_sb:
                        load_w(wname)
                    tiles = [load_input(name, kt) for kt in range(KT8)]
                    for s in range(NTC // 2):
                        ps = prps.tile([128, 2, TC], F32, tag="pr")
                        for kt in range(KT8):
                            lw = w_sb[wname][:, kt, pair * 128:(pair + 1) * 128]
                            for i in range(2):
                                tc0 = (2 * s + i) * TC
                                nc.tensor.matmul(
                                    ps[:, i, :], lw, tiles[kt][:, tc0:tc0 + TC],
                                    start=(kt == 0), stop=(kt == KT8 - 1))
                        nc.vector.tensor_scalar_add(
                            dst[:, pair, 2 * s * TC:(2 * s + 2) * TC],
                            ps[:].rearrange("p a b -> p (a b)"),
                            bias[:, pair:pair + 1])

            def proj_v():
                load_w("wv")
                nc.vector.memset(V_aug[:, :, :, 64:65], 1.0)
                tiles = [load_input("v", kt) for kt in range(KT8)]
                for s in range(NTT // 2):
                    ps = prps.tile([128, 2, JW], F32, tag="pr")
                    for kt in range(KT8):
                        for i in range(2):
                            tt = 2 * s + i
                            nc.tensor.matmul(
                                ps[:, i, :],
                                tiles[kt][:, tt * 128:(tt + 1) * 128],
                                w_sb["wv"][:, kt, :],
                                start=(kt == 0), stop=(kt == KT8 - 1))
                    nc.vector.tensor_copy(
                        V_aug[:, 2 * s:2 * s + 2, :, 0:64],
                        ps[:].rearrange("p a (h d) -> p a h d", d=64))

            def attn_scores(pair, qt):
                """Emit 16 (scores, exp) groups; return the et tiles."""
                q0 = qt * qt_size
                ets = []
                for kt in range(NKT):
                    sc = scps.tile([128, 2, qt_size], F32, tag="sc")
                    for h2 in range(2):
                        nc.tensor.matmul(
                            sc[:, h2, :],
                            KT_sb[h2 * 64:(h2 + 1) * 64, pair,
                                  kt * 128:(kt + 1) * 128],
                            QT_sb[h2 * 64:(h2 + 1) * 64, pair, q0:q0 + qt_size],
                            start=True, stop=True)
                    et = exps.tile([128, 2, qt_size], BF16, tag="exp")
                    nc.scalar.activation(
                        et[:].rearrange("p a b -> p (a b)"),
                        sc[:].rearrange("p a b -> p (a b)"),
                        mybir.ActivationFunctionType.Exp, scale=0.125)
                    ets.append(et)
                return ets

            def attn_pv(pair, qt, ets):
                """Trailing per-head PV chains (1 PSUM bank each, bufs=2)."""
                q0 = qt * qt_size
                for h2 in range(2):
                    h = pair * 2 + h2
                    pv = pvps.tile([65, qt_size], F32, tag="pv")
                    for kt in range(NKT):
                        nc.tensor.matmul(
                            pv[:],
                            V_aug[:, kt, h, :],
                            ets[kt][:, h2, :],
                            start=(kt == 0), stop=(kt == NKT - 1))
                    ot = ostage.tile([65, qt_size], F32, tag="ot")
                    nc.vector.tensor_copy(ot[:], pv[:])
                    nc.sync.dma_start(
                        out=numT.ap()[h * 64:(h + 1) * 64, q0:q0 + qt_size],
                        in_=ot[0:64, :])
                    nc.sync.dma_start(
                        out=den.ap()[h:h + 1, q0:q0 + qt_size],
                        in_=ot[64:65, :])

            def attn_qt(pair, qt):
                attn_pv(pair, qt, attn_scores(pair, qt))

            load_w("wk")
            bias_q = load_bias("bq", bq)
            bias_k = load_bias("bk", bk)
            proj_qk(0)
            ets0 = attn_scores(0, 0)
            proj_v()
            attn_pv(0, 0, ets0)
            for qt in range(1, NQT):
                attn_qt(0, qt)
            for pair in range(1, NP):
                proj_qk(pair)
                for qt in range(NQT):
                    attn_qt(pair, qt)

    nc.compile()
    return nc


_NC_CACHE = {}


def _get_nc():
    if "nc" not in _NC_CACHE:
        _NC_CACHE["nc"] = _build_nc()
    return _NC_CACHE["nc"]


def _make_in_maps(key, value, query, Wq, bq, Wk, bk, Wv):
    in_maps = []
    for c in range(N_CORES):
        b, hh = c // 2, c % 2
        js = slice(hh * JW, (hh + 1) * JW)
        in_maps.append({
            "qT": np.ascontiguousarray(query[b].T).astype(_BF),
            "kT": np.ascontiguousarray(key[b].T).astype(_BF),
            "vT": np.ascontiguousarray(value[b].T).astype(_BF),
            "wq": np.ascontiguousarray(Wq[:, js]).astype(_BF),
            "wk": np.ascontiguousarray(Wk[:, js]).astype(_BF),
            "wv": np.ascontiguousarray(Wv[:, js]).astype(_BF),
            "bq": np.ascontiguousarray(bq[js], dtype=np.float32),
            "bk": np.ascontiguousarray(bk[js], dtype=np.float32),
        })
    return in_maps


def _assemble(results, bv):
    out = np.empty((B, S, H * DH), np.float32)
    for c in range(N_CORES):
        b, hh = c // 2, c % 2
        numT = results[c]["numT"]
        den = results[c]["den"]
        blk = numT.reshape(HH, DH, S) / den[:, None, :]
        out[b, :, hh * JW:(hh + 1) * JW] = (
            blk.reshape(JW, S).T + bv[hh * JW:(hh + 1) * JW])
    return out


def kernel(key, value, query, Wq, bq, Wk, bk, Wv, bv, **_run_kwargs):
    key = np.asarray(key, np.float32)
    value = np.asarray(value, np.float32)
    query = np.asarray(query, np.float32)
    nc = _get_nc()
    in_maps = _make_in_maps(key, value, query,
                            np.asarray(Wq, np.float32), np.asarray(bq, np.float32),
                            np.asarray(Wk, np.float32), np.asarray(bk, np.float32),
                            np.asarray(Wv, np.float32))
    res = run_bass_kernel_spmd(nc, in_maps, list(range(N_CORES)), **_run_kwargs)
    out = _assemble(res.results, np.asarray(bv, np.float32))
    if _run_kwargs:
        kernel.last_result = res
    return out
